# revision 10
# baseline (speedup 1.0000x reference)
"""Trainium2 Bass kernel for nn_Blur: depthwise 4x4 FIR blur (upfirdn2d pad=(2,1)).

Full inputs in, full output out. Internally shards the 4096 (b,c) images
across 8 NeuronCores (pure data parallel, no collectives).

Algorithm (per core, 512 images of [H=128, W=128]):
  out[ho, wo] = sum_{i,j} wf[i, j] * x[ho+i-2, wo+j-2]   (wf = flipped kernel)
which factors into 4 column-convolutions along H, each a banded matmul with
the contraction over the partition (H) axis, with the W-shift (j-2) realized
as a shifted PSUM write of an accumulating matmul:
  psum[:, c] += W_j^T @ x[:, c + (j-2)]     W_j[hi, ho] = wf[hi-ho+2, j]
Images are packed 3-per-PSUM-bank with 2-column zero gaps so the shifted
reads pick up zero padding at image edges and the moving free dim (391)
stays >= 256, where float32r matmuls run at 1 cycle/row.
"""

import os
import sys
from contextlib import ExitStack

for _p in ("/opt/trn_rl_repo", "/root/.axon_site/_ro/trn_rl_repo"):
    if os.path.isdir(_p) and _p not in sys.path:
        sys.path.append(_p)

import numpy as np

import concourse.bass as bass  # noqa: F401  (engine types referenced via nc)
import concourse.tile as tile
from concourse import bacc, bass_utils, mybir

B, C, H, W = 16, 256, 128, 128
N_CORES = 8
GROUP = 3          # images per PSUM bank / matmul group
STRIDE = 130       # 2-col gap + 128 data cols per image in the packed tile
PAD0 = 2           # upfirdn2d pad before (both spatial dims)

_PROGRAM_CACHE: dict[int, object] = {}


def _round_fp32r(a: np.ndarray) -> np.ndarray:
    """Round fp32 to fp32r (11-bit mantissa, RNE) — matches neuronxcc's
    static_cast_fp32_to_fp32r bit-exactly."""
    u = np.ascontiguousarray(a, dtype=np.float32).view(np.uint32)
    r = (u.astype(np.uint64) + 0x800 - ((u >> 12) & 1)) & 0xFFFFF000
    return r.astype(np.uint32).view(np.float32)


def _band_matrices(kern: np.ndarray) -> np.ndarray:
    """bands[j][hi, ho] = wf[hi-ho+2, j], wf = flip(kern). Shape [4,128,128]."""
    wf = np.flip(np.asarray(kern, dtype=np.float64), (0, 1))
    bands = np.zeros((4, H, H), dtype=np.float64)
    ho = np.arange(H)
    for j in range(4):
        for i in range(4):
            d = i - PAD0            # hi - ho
            hi = ho + d
            m = (hi >= 0) & (hi < H)
            bands[j][hi[m], ho[m]] = wf[i, j]
    return np.ascontiguousarray(bands.astype(np.float32))


def _groups(n_images: int):
    out = []
    i = 0
    while i < n_images:
        n = min(GROUP, n_images - i)
        out.append((i, n))
        i += n
    # avoid a trailing 1-image group (moving dim 131 < 256 is 4x slower):
    # rebalance the last two groups 3+1 -> 2+2
    if len(out) >= 2 and out[-1][1] == 1:
        i0, n0 = out[-2]
        out[-2] = (i0, 2)
        out[-1] = (i0 + 2, 2)
    return out


def build_program(n_images: int, xt_bufs: int = 4):
    """Build + compile the per-core Bass program for n_images [128,128] images."""
    nc = bacc.Bacc("TRN2", target_bir_lowering=False, debug=False)
    f32 = mybir.dt.float32
    f32r = mybir.dt.float32r

    x_d = nc.dram_tensor("x", [n_images, H, W], f32r, kind="ExternalInput")
    b_d = nc.dram_tensor("bands", [4, H, H], f32r, kind="ExternalInput")
    z_d = nc.dram_tensor("zeros", [H, 2 * GROUP + 2], f32r, kind="ExternalInput")
    y_d = nc.dram_tensor("y", [n_images, H, W], f32, kind="ExternalOutput")

    wtot3 = STRIDE * GROUP + 2  # even width; cols {130k, 130k+1} are zero gaps

    with ExitStack() as ctx:
        tc = ctx.enter_context(tile.TileContext(nc))
        wpool = ctx.enter_context(tc.tile_pool(name="wpool", bufs=1))
        xpool = ctx.enter_context(tc.tile_pool(name="xpool", bufs=1))
        opool = ctx.enter_context(tc.tile_pool(name="opool", bufs=4))
        ppool = ctx.enter_context(tc.tile_pool(name="ppool", bufs=4, space="PSUM"))

        wt = wpool.tile([H, 4 * H], f32r)
        nc.sync.dma_start(
            wt.rearrange("p (j b) -> p j b", b=H), b_d.rearrange("j a b -> a j b")
        )

        # Persistent input tiles: gap columns are zeroed ONCE via DMA from the
        # zeros input; per-group DMAs only ever write data columns, so the
        # zero padding between images survives tile reuse. (A memset would be
        # simpler but fp32r memset fails walrus codegen / crashes the engine.)
        xts = []
        for k in range(xt_bufs):
            xt = xpool.tile([H, wtot3], f32r, name=f"xt{k}", tag=f"xt{k}")
            gaps = xt[:, 0 : STRIDE * GROUP].rearrange("p (k c) -> p k c", c=STRIDE)
            nc.sync.dma_start(
                gaps[:, :, 0:PAD0],
                z_d[:, 0 : 2 * GROUP].rearrange("p (k c) -> p k c", c=PAD0),
            )
            nc.sync.dma_start(xt[:, STRIDE * GROUP : wtot3], z_d[:, 6:8])
            xts.append(xt)

        for g, (i0, n) in enumerate(_groups(n_images)):
            wtot = STRIDE * n + 2
            xt = xts[g % xt_bufs]
            for k in range(n):
                nc.sync.dma_start(
                    xt[:, STRIDE * k + PAD0 : STRIDE * k + PAD0 + W],
                    x_d[i0 + k],
                )

            # fp32r matmul ISA restrictions: dst start col even (8B-aligned)
            # and dst/src innermost lengths even. dst [2, 130n+2) (j<=2) /
            # [2, 130n) (j=3); extra columns land in zero-gap / never-read
            # psum columns, so correctness is unaffected.
            pt = ppool.tile([H, wtot], f32, tag="pt")
            for idx, j in enumerate((2, 0, 1, 3)):  # full-width write first
                d = j - PAD0
                a = PAD0
                b = STRIDE * n + PAD0 - (PAD0 if d > 0 else 0)
                nc.tensor.matmul(
                    pt[:, a:b],
                    wt[:, H * j : H * (j + 1)],
                    xt[:, a + d : b + d],
                    start=(idx == 0),
                    stop=(idx == 3),
                )

            ot = opool.tile([H, n * W], f32, tag="ot")
            psrc = pt[:, 0 : STRIDE * n].rearrange("p (k c) -> p k c", c=STRIDE)
            odst = ot.rearrange("p (k c) -> p k c", c=W)
            if g % 2 == 0:
                nc.vector.tensor_copy(odst, psrc[:, :, PAD0 : PAD0 + W])
            else:
                nc.scalar.copy(odst, psrc[:, :, PAD0 : PAD0 + W])
            for k in range(n):
                nc.sync.dma_start(y_d[i0 + k], ot[:, W * k : W * (k + 1)])

    nc.compile()
    return nc


def _get_program(n_images: int):
    if n_images not in _PROGRAM_CACHE:
        _PROGRAM_CACHE[n_images] = build_program(n_images)
    return _PROGRAM_CACHE[n_images]


def kernel(x: np.ndarray, kernel: np.ndarray, _trace: bool = False):
    x = np.ascontiguousarray(x, dtype=np.float32)
    assert x.shape == (B, C, H, W), x.shape
    bands = _band_matrices(kernel)

    n_total = B * C
    n_per_core = n_total // N_CORES
    xf = x.reshape(n_total, H, W)

    nc = _get_program(n_per_core)
    in_maps = [
        {
            "x": _round_fp32r(xf[c * n_per_core : (c + 1) * n_per_core]),
            "bands": _round_fp32r(bands),
            "zeros": np.zeros((H, 2 * GROUP + 2), dtype=np.float32),
        }
        for c in range(N_CORES)
    ]
    res = bass_utils.run_bass_kernel_spmd(
        nc, in_maps, core_ids=list(range(N_CORES)), trace=_trace
    )
    y = np.concatenate([r["y"] for r in res.results], axis=0).reshape(B, C, H, W)
    if _trace:
        return y, res
    return y


# revision 13
# speedup vs baseline: 2.6551x; 2.6551x over previous
"""Trainium2 Bass kernel for nn_Blur: depthwise 4x4 FIR blur (upfirdn2d pad=(2,1)).

Full inputs in, full output out. Internally shards the 4096 (b,c) images
across 8 NeuronCores (pure data parallel, no collectives).

Algorithm (per core, 512 images of [H=128, W=128]):
  out[ho, wo] = sum_{i,j} wf[i, j] * x[ho+i-2, wo+j-2]   (wf = flipped kernel)
which factors into 4 column-convolutions along H, each a banded matmul with
the contraction over the partition (H) axis, with the W-shift (j-2) realized
as a shifted PSUM write of an accumulating matmul:
  psum[:, c] += W_j^T @ x[:, c + (j-2)]     W_j[hi, ho] = wf[hi-ho+2, j]
Images are packed 3-per-PSUM-bank with 2-column zero gaps so the shifted
reads pick up zero padding at image edges and the moving free dim (391)
stays >= 256, where float32r matmuls run at 1 cycle/row.
"""

import os
import sys
from contextlib import ExitStack

for _p in ("/opt/trn_rl_repo", "/root/.axon_site/_ro/trn_rl_repo"):
    if os.path.isdir(_p) and _p not in sys.path:
        sys.path.append(_p)

import numpy as np

import concourse.bass as bass  # noqa: F401  (engine types referenced via nc)
import concourse.tile as tile
from concourse import bacc, bass_utils, mybir

B, C, H, W = 16, 256, 128, 128
N_CORES = 8
GROUP = 3          # images per PSUM bank / matmul group
STRIDE = 130       # 2-col gap + 128 data cols per image in the packed tile
PAD0 = 2           # upfirdn2d pad before (both spatial dims)

_PROGRAM_CACHE: dict[int, object] = {}


def _round_fp32r(a: np.ndarray) -> np.ndarray:
    """Round fp32 to fp32r (11-bit mantissa, RNE) — matches neuronxcc's
    static_cast_fp32_to_fp32r bit-exactly."""
    u = np.ascontiguousarray(a, dtype=np.float32).view(np.uint32)
    r = (u.astype(np.uint64) + 0x800 - ((u >> 12) & 1)) & 0xFFFFF000
    return r.astype(np.uint32).view(np.float32)


def _band_matrices(kern: np.ndarray) -> np.ndarray:
    """bands[j][hi, ho] = wf[hi-ho+2, j], wf = flip(kern). Shape [4,128,128]."""
    wf = np.flip(np.asarray(kern, dtype=np.float64), (0, 1))
    bands = np.zeros((4, H, H), dtype=np.float64)
    ho = np.arange(H)
    for j in range(4):
        for i in range(4):
            d = i - PAD0            # hi - ho
            hi = ho + d
            m = (hi >= 0) & (hi < H)
            bands[j][hi[m], ho[m]] = wf[i, j]
    return np.ascontiguousarray(bands.astype(np.float32))


def _groups(n_images: int):
    out = []
    i = 0
    while i < n_images:
        n = min(GROUP, n_images - i)
        out.append((i, n))
        i += n
    # avoid a trailing 1-image group (moving dim 131 < 256 is 4x slower):
    # rebalance the last two groups 3+1 -> 2+2
    if len(out) >= 2 and out[-1][1] == 1:
        i0, n0 = out[-2]
        out[-2] = (i0, 2)
        out[-1] = (i0 + 2, 2)
    return out


def build_program(n_images: int, xt_bufs: int = 8, qg: int = 4):
    """Build + compile the per-core Bass program for n_images [128,128] images.

    qg = PSUM banks in flight per weight-batched supergroup (j-outer order
    keeps PE matmuls dense); DMAs round-robin across engine DGE rings.
    """
    nc = bacc.Bacc("TRN2", target_bir_lowering=False, debug=False)
    f32 = mybir.dt.float32
    f32r = mybir.dt.float32r

    x_d = nc.dram_tensor("x", [n_images, H, W], f32r, kind="ExternalInput")
    b_d = nc.dram_tensor("bands", [4, H, H], f32r, kind="ExternalInput")
    z_d = nc.dram_tensor("zeros", [H, 2 * GROUP + 2], f32r, kind="ExternalInput")
    y_d = nc.dram_tensor("y", [n_images, H, W], f32, kind="ExternalOutput")

    wtot3 = STRIDE * GROUP + 2  # even width; cols {130k, 130k+1} are zero gaps

    with ExitStack() as ctx:
        tc = ctx.enter_context(tile.TileContext(nc))
        wpool = ctx.enter_context(tc.tile_pool(name="wpool", bufs=1))
        xpool = ctx.enter_context(tc.tile_pool(name="xpool", bufs=1))
        opool = ctx.enter_context(tc.tile_pool(name="opool", bufs=6))
        ppool = ctx.enter_context(tc.tile_pool(name="ppool", bufs=8, space="PSUM"))

        dma_engines = [nc.sync, nc.scalar, nc.gpsimd]

        wt = wpool.tile([H, 4 * H], f32r)
        nc.sync.dma_start(
            wt.rearrange("p (j b) -> p j b", b=H), b_d.rearrange("j a b -> a j b")
        )

        # Persistent input tiles: gap columns are zeroed ONCE via DMA from the
        # zeros input; per-group DMAs only ever write data columns, so the
        # zero padding between images survives tile reuse. (A memset would be
        # simpler but fp32r memset fails walrus codegen / crashes the engine.)
        xts = []
        for k in range(xt_bufs):
            xt = xpool.tile([H, wtot3], f32r, name=f"xt{k}", tag=f"xt{k}")
            gaps = xt[:, 0 : STRIDE * GROUP].rearrange("p (k c) -> p k c", c=STRIDE)
            nc.sync.dma_start(
                gaps[:, :, 0:PAD0],
                z_d[:, 0 : 2 * GROUP].rearrange("p (k c) -> p k c", c=PAD0),
            )
            nc.sync.dma_start(xt[:, STRIDE * GROUP : wtot3], z_d[:, 6:8])
            xts.append(xt)

        gs = _groups(n_images)
        for s in range(0, len(gs), qg):
            chunk = gs[s : s + qg]
            pts = []
            for q, (i0, n) in enumerate(chunk):
                g = s + q
                xt = xts[g % xt_bufs]
                # one batched input DMA per group: DRAM side sequential
                dst = xt[:, 0 : STRIDE * n].rearrange("p (k c) -> p k c", c=STRIDE)
                dma_engines[g % len(dma_engines)].dma_start(
                    dst[:, :, PAD0 : PAD0 + W],
                    x_d[i0 : i0 + n].rearrange("n h w -> h n w"),
                )
                pts.append(ppool.tile([H, STRIDE * n + 2], f32, tag="pt", name="pt"))

            # fp32r matmul ISA restrictions: dst start col even (8B-aligned)
            # and dst/src innermost lengths even. dst [2, 130n+2) (j<=2) /
            # [2, 130n) (j=3); extra columns land in zero-gap / never-read
            # psum columns, so correctness is unaffected. j-outer order so
            # the 4 weight loads amortize over qg matmuls each.
            for idx, j in enumerate((2, 0, 1, 3)):  # full-width write first
                d = j - PAD0
                for q, (i0, n) in enumerate(chunk):
                    a = PAD0
                    b = STRIDE * n + PAD0 - (PAD0 if d > 0 else 0)
                    nc.tensor.matmul(
                        pts[q][:, a:b],
                        wt[:, H * j : H * (j + 1)],
                        xts[(s + q) % xt_bufs][:, a + d : b + d],
                        start=(idx == 0),
                        stop=(idx == 3),
                    )

            for q, (i0, n) in enumerate(chunk):
                g = s + q
                pt = pts[q]
                ot = opool.tile([H, n * W], f32, tag="ot", name="ot")
                psrc = pt[:, 0 : STRIDE * n].rearrange("p (k c) -> p k c", c=STRIDE)
                odst = ot.rearrange("p (k c) -> p k c", c=W)
                if g % 2 == 0:
                    nc.vector.tensor_copy(odst, psrc[:, :, PAD0 : PAD0 + W])
                else:
                    nc.scalar.copy(odst, psrc[:, :, PAD0 : PAD0 + W])
                osrc = ot.rearrange("p (k c) -> p k c", c=W)
                dma_engines[(g + 1) % len(dma_engines)].dma_start(
                    y_d[i0 : i0 + n].rearrange("n h w -> h n w"), osrc
                )

    nc.compile()
    return nc


def _get_program(n_images: int):
    if n_images not in _PROGRAM_CACHE:
        _PROGRAM_CACHE[n_images] = build_program(n_images)
    return _PROGRAM_CACHE[n_images]


def kernel(x: np.ndarray, kernel: np.ndarray, _trace: bool = False):
    x = np.ascontiguousarray(x, dtype=np.float32)
    assert x.shape == (B, C, H, W), x.shape
    bands = _band_matrices(kernel)

    n_total = B * C
    n_per_core = n_total // N_CORES
    xf = x.reshape(n_total, H, W)

    nc = _get_program(n_per_core)
    in_maps = [
        {
            "x": _round_fp32r(xf[c * n_per_core : (c + 1) * n_per_core]),
            "bands": _round_fp32r(bands),
            "zeros": np.zeros((H, 2 * GROUP + 2), dtype=np.float32),
        }
        for c in range(N_CORES)
    ]
    res = bass_utils.run_bass_kernel_spmd(
        nc, in_maps, core_ids=list(range(N_CORES)), trace=_trace
    )
    y = np.concatenate([r["y"] for r in res.results], axis=0).reshape(B, C, H, W)
    if _trace:
        return y, res
    return y


# revision 14
# speedup vs baseline: 2.7312x; 1.0287x over previous
"""Trainium2 Bass kernel for nn_Blur: depthwise 4x4 FIR blur (upfirdn2d pad=(2,1)).

Full inputs in, full output out. Internally shards the 4096 (b,c) images
across 8 NeuronCores (pure data parallel, no collectives).

Algorithm (per core, 512 images of [H=128, W=128]):
  out[ho, wo] = sum_{i,j} wf[i, j] * x[ho+i-2, wo+j-2]   (wf = flipped kernel)
which factors into 4 column-convolutions along H, each a banded matmul with
the contraction over the partition (H) axis, with the W-shift (j-2) realized
as a shifted PSUM write of an accumulating matmul:
  psum[:, c] += W_j^T @ x[:, c + (j-2)]     W_j[hi, ho] = wf[hi-ho+2, j]
Images are packed 3-per-PSUM-bank with 2-column zero gaps so the shifted
reads pick up zero padding at image edges and the moving free dim (391)
stays >= 256, where float32r matmuls run at 1 cycle/row.
"""

import os
import sys
from contextlib import ExitStack

for _p in ("/opt/trn_rl_repo", "/root/.axon_site/_ro/trn_rl_repo"):
    if os.path.isdir(_p) and _p not in sys.path:
        sys.path.append(_p)

import numpy as np

import concourse.bass as bass  # noqa: F401  (engine types referenced via nc)
import concourse.tile as tile
from concourse import bacc, bass_utils, mybir

B, C, H, W = 16, 256, 128, 128
N_CORES = 8
GROUP = 3          # images per PSUM bank / matmul group
STRIDE = 130       # 2-col gap + 128 data cols per image in the packed tile
PAD0 = 2           # upfirdn2d pad before (both spatial dims)

_PROGRAM_CACHE: dict[int, object] = {}


def _round_fp32r(a: np.ndarray) -> np.ndarray:
    """Round fp32 to fp32r (11-bit mantissa, RNE) — matches neuronxcc's
    static_cast_fp32_to_fp32r bit-exactly."""
    u = np.ascontiguousarray(a, dtype=np.float32).view(np.uint32)
    r = (u.astype(np.uint64) + 0x800 - ((u >> 12) & 1)) & 0xFFFFF000
    return r.astype(np.uint32).view(np.float32)


def _band_matrices(kern: np.ndarray) -> np.ndarray:
    """bands[j][hi, ho] = wf[hi-ho+2, j], wf = flip(kern). Shape [4,128,128]."""
    wf = np.flip(np.asarray(kern, dtype=np.float64), (0, 1))
    bands = np.zeros((4, H, H), dtype=np.float64)
    ho = np.arange(H)
    for j in range(4):
        for i in range(4):
            d = i - PAD0            # hi - ho
            hi = ho + d
            m = (hi >= 0) & (hi < H)
            bands[j][hi[m], ho[m]] = wf[i, j]
    return np.ascontiguousarray(bands.astype(np.float32))


def _groups(n_images: int):
    out = []
    i = 0
    while i < n_images:
        n = min(GROUP, n_images - i)
        out.append((i, n))
        i += n
    # avoid a trailing 1-image group (moving dim 131 < 256 is 4x slower):
    # rebalance the last two groups 3+1 -> 2+2
    if len(out) >= 2 and out[-1][1] == 1:
        i0, n0 = out[-2]
        out[-2] = (i0, 2)
        out[-1] = (i0 + 2, 2)
    return out


def build_program(n_images: int, xt_bufs: int = 12, qg: int = 4):
    """Build + compile the per-core Bass program for n_images [128,128] images.

    qg = PSUM banks in flight per weight-batched supergroup (j-outer order
    keeps PE matmuls dense); DMAs round-robin across engine DGE rings.
    """
    nc = bacc.Bacc("TRN2", target_bir_lowering=False, debug=False)
    f32 = mybir.dt.float32
    f32r = mybir.dt.float32r

    x_d = nc.dram_tensor("x", [n_images, H, W], f32r, kind="ExternalInput")
    b_d = nc.dram_tensor("bands", [4, H, H], f32r, kind="ExternalInput")
    z_d = nc.dram_tensor("zeros", [H, 2 * GROUP + 2], f32r, kind="ExternalInput")
    y_d = nc.dram_tensor("y", [n_images, H, W], f32, kind="ExternalOutput")

    wtot3 = STRIDE * GROUP + 2  # even width; cols {130k, 130k+1} are zero gaps

    with ExitStack() as ctx:
        tc = ctx.enter_context(tile.TileContext(nc))
        wpool = ctx.enter_context(tc.tile_pool(name="wpool", bufs=1))
        xpool = ctx.enter_context(tc.tile_pool(name="xpool", bufs=1))
        opool = ctx.enter_context(tc.tile_pool(name="opool", bufs=8))
        ppool = ctx.enter_context(tc.tile_pool(name="ppool", bufs=8, space="PSUM"))

        dma_engines = [nc.sync, nc.scalar, nc.gpsimd]

        wt = wpool.tile([H, 4 * H], f32r)
        nc.sync.dma_start(
            wt.rearrange("p (j b) -> p j b", b=H), b_d.rearrange("j a b -> a j b")
        )

        # Persistent input tiles: gap columns are zeroed ONCE via DMA from the
        # zeros input; per-group DMAs only ever write data columns, so the
        # zero padding between images survives tile reuse. (A memset would be
        # simpler but fp32r memset fails walrus codegen / crashes the engine.)
        xts = []
        for k in range(xt_bufs):
            xt = xpool.tile([H, wtot3], f32r, name=f"xt{k}", tag=f"xt{k}")
            gaps = xt[:, 0 : STRIDE * GROUP].rearrange("p (k c) -> p k c", c=STRIDE)
            nc.sync.dma_start(
                gaps[:, :, 0:PAD0],
                z_d[:, 0 : 2 * GROUP].rearrange("p (k c) -> p k c", c=PAD0),
            )
            nc.sync.dma_start(xt[:, STRIDE * GROUP : wtot3], z_d[:, 6:8])
            xts.append(xt)

        gs = _groups(n_images)
        chunks = [gs[s : s + qg] for s in range(0, len(gs), qg)]

        def emit_in_dmas(ci):
            for q, (i0, n) in enumerate(chunks[ci]):
                g = ci * qg + q
                xt = xts[g % xt_bufs]
                # one batched input DMA per group
                dst = xt[:, 0 : STRIDE * n].rearrange("p (k c) -> p k c", c=STRIDE)
                dma_engines[g % len(dma_engines)].dma_start(
                    dst[:, :, PAD0 : PAD0 + W],
                    x_d[i0 : i0 + n].rearrange("n h w -> h n w"),
                )

        emit_in_dmas(0)
        for ci, chunk in enumerate(chunks):
            # software-pipelined emission: prefetch next supergroup's inputs
            # before this supergroup's matmuls so PE never starves (HAM stays
            # warm).
            if ci + 1 < len(chunks):
                emit_in_dmas(ci + 1)

            pts = [
                ppool.tile([H, STRIDE * n + 2], f32, tag="pt", name="pt")
                for (i0, n) in chunk
            ]
            # fp32r matmul ISA restrictions: dst start col even (8B-aligned)
            # and dst/src innermost lengths even. dst [2, 130n+2) (j<=2) /
            # [2, 130n) (j=3); extra columns land in zero-gap / never-read
            # psum columns, so correctness is unaffected. j-outer order so
            # the 4 weight loads amortize over qg matmuls each.
            for idx, j in enumerate((2, 0, 1, 3)):  # full-width write first
                d = j - PAD0
                for q, (i0, n) in enumerate(chunk):
                    a = PAD0
                    b = STRIDE * n + PAD0 - (PAD0 if d > 0 else 0)
                    nc.tensor.matmul(
                        pts[q][:, a:b],
                        wt[:, H * j : H * (j + 1)],
                        xts[(ci * qg + q) % xt_bufs][:, a + d : b + d],
                        start=(idx == 0),
                        stop=(idx == 3),
                    )

            for q, (i0, n) in enumerate(chunk):
                g = ci * qg + q
                pt = pts[q]
                ot = opool.tile([H, n * W], f32, tag="ot", name="ot")
                psrc = pt[:, 0 : STRIDE * n].rearrange("p (k c) -> p k c", c=STRIDE)
                odst = ot.rearrange("p (k c) -> p k c", c=W)
                if g % 2 == 0:
                    nc.vector.tensor_copy(odst, psrc[:, :, PAD0 : PAD0 + W])
                else:
                    nc.scalar.copy(odst, psrc[:, :, PAD0 : PAD0 + W])
                osrc = ot.rearrange("p (k c) -> p k c", c=W)
                dma_engines[(g + 1) % len(dma_engines)].dma_start(
                    y_d[i0 : i0 + n].rearrange("n h w -> h n w"), osrc
                )

    nc.compile()
    return nc


def _get_program(n_images: int):
    if n_images not in _PROGRAM_CACHE:
        _PROGRAM_CACHE[n_images] = build_program(n_images)
    return _PROGRAM_CACHE[n_images]


def kernel(x: np.ndarray, kernel: np.ndarray, _trace: bool = False):
    x = np.ascontiguousarray(x, dtype=np.float32)
    assert x.shape == (B, C, H, W), x.shape
    bands = _band_matrices(kernel)

    n_total = B * C
    n_per_core = n_total // N_CORES
    xf = x.reshape(n_total, H, W)

    nc = _get_program(n_per_core)
    in_maps = [
        {
            "x": _round_fp32r(xf[c * n_per_core : (c + 1) * n_per_core]),
            "bands": _round_fp32r(bands),
            "zeros": np.zeros((H, 2 * GROUP + 2), dtype=np.float32),
        }
        for c in range(N_CORES)
    ]
    res = bass_utils.run_bass_kernel_spmd(
        nc, in_maps, core_ids=list(range(N_CORES)), trace=_trace
    )
    y = np.concatenate([r["y"] for r in res.results], axis=0).reshape(B, C, H, W)
    if _trace:
        return y, res
    return y


# revision 15
# speedup vs baseline: 2.8822x; 1.0553x over previous
"""Trainium2 Bass kernel for nn_Blur: depthwise 4x4 FIR blur (upfirdn2d pad=(2,1)).

Full inputs in, full output out. Internally shards the 4096 (b,c) images
across 8 NeuronCores (pure data parallel, no collectives).

Algorithm (per core, 512 images of [H=128, W=128]):
  out[ho, wo] = sum_{i,j} wf[i, j] * x[ho+i-2, wo+j-2]   (wf = flipped kernel)
which factors into 4 column-convolutions along H, each a banded matmul with
the contraction over the partition (H) axis, with the W-shift (j-2) realized
as a shifted PSUM write of an accumulating matmul:
  psum[:, c] += W_j^T @ x[:, c + (j-2)]     W_j[hi, ho] = wf[hi-ho+2, j]
Images are packed 3-per-PSUM-bank with 2-column zero gaps so the shifted
reads pick up zero padding at image edges and the moving free dim (391)
stays >= 256, where float32r matmuls run at 1 cycle/row.
"""

import os
import sys
from contextlib import ExitStack

for _p in ("/opt/trn_rl_repo", "/root/.axon_site/_ro/trn_rl_repo"):
    if os.path.isdir(_p) and _p not in sys.path:
        sys.path.append(_p)

import numpy as np

import concourse.bass as bass  # noqa: F401  (engine types referenced via nc)
import concourse.tile as tile
from concourse import bacc, bass_utils, mybir

B, C, H, W = 16, 256, 128, 128
N_CORES = 8
GROUP = 3          # images per PSUM bank / matmul group
STRIDE = 130       # 2-col gap + 128 data cols per image in the packed tile
PAD0 = 2           # upfirdn2d pad before (both spatial dims)

_PROGRAM_CACHE: dict[int, object] = {}


def _round_fp32r(a: np.ndarray) -> np.ndarray:
    """Round fp32 to fp32r (11-bit mantissa, RNE) — matches neuronxcc's
    static_cast_fp32_to_fp32r bit-exactly."""
    u = np.ascontiguousarray(a, dtype=np.float32).view(np.uint32)
    r = (u.astype(np.uint64) + 0x800 - ((u >> 12) & 1)) & 0xFFFFF000
    return r.astype(np.uint32).view(np.float32)


def _band_matrices(kern: np.ndarray) -> np.ndarray:
    """bands[j][hi, ho] = wf[hi-ho+2, j], wf = flip(kern). Shape [4,128,128]."""
    wf = np.flip(np.asarray(kern, dtype=np.float64), (0, 1))
    bands = np.zeros((4, H, H), dtype=np.float64)
    ho = np.arange(H)
    for j in range(4):
        for i in range(4):
            d = i - PAD0            # hi - ho
            hi = ho + d
            m = (hi >= 0) & (hi < H)
            bands[j][hi[m], ho[m]] = wf[i, j]
    return np.ascontiguousarray(bands.astype(np.float32))


def _groups(n_images: int):
    out = []
    i = 0
    while i < n_images:
        n = min(GROUP, n_images - i)
        out.append((i, n))
        i += n
    # avoid a trailing 1-image group (moving dim 131 < 256 is 4x slower):
    # rebalance the last two groups 3+1 -> 2+2
    if len(out) >= 2 and out[-1][1] == 1:
        i0, n0 = out[-2]
        out[-2] = (i0, 2)
        out[-1] = (i0 + 2, 2)
    return out


def build_program(n_images: int, xt_bufs: int = 12, qg: int = 4):
    """Build + compile the per-core Bass program for n_images [128,128] images.

    qg = PSUM banks in flight per weight-batched supergroup (j-outer order
    keeps PE matmuls dense); DMAs round-robin across engine DGE rings.
    """
    nc = bacc.Bacc("TRN2", target_bir_lowering=False, debug=False)
    f32 = mybir.dt.float32
    f32r = mybir.dt.float32r

    x_d = nc.dram_tensor("x", [n_images, H, W], f32r, kind="ExternalInput")
    b_d = nc.dram_tensor("bands", [4, H, H], f32r, kind="ExternalInput")
    z_d = nc.dram_tensor("zeros", [H, 2 * GROUP + 2], f32r, kind="ExternalInput")
    y_d = nc.dram_tensor("y", [n_images, H, W], f32, kind="ExternalOutput")

    wtot3 = STRIDE * GROUP + 2  # even width; cols {130k, 130k+1} are zero gaps

    with ExitStack() as ctx:
        tc = ctx.enter_context(tile.TileContext(nc))
        wpool = ctx.enter_context(tc.tile_pool(name="wpool", bufs=1))
        xpool = ctx.enter_context(tc.tile_pool(name="xpool", bufs=1))
        opool = ctx.enter_context(tc.tile_pool(name="opool", bufs=8))
        ppool = ctx.enter_context(tc.tile_pool(name="ppool", bufs=8, space="PSUM"))

        dma_engines = [nc.sync, nc.scalar, nc.gpsimd]

        wt = wpool.tile([H, 4 * H], f32r)
        nc.sync.dma_start(
            wt.rearrange("p (j b) -> p j b", b=H), b_d.rearrange("j a b -> a j b")
        )

        # Persistent input tiles: gap columns are zeroed ONCE via DMA from the
        # zeros input; per-group DMAs only ever write data columns, so the
        # zero padding between images survives tile reuse. (A memset would be
        # simpler but fp32r memset fails walrus codegen / crashes the engine.)
        xts = []
        for k in range(xt_bufs):
            xt = xpool.tile([H, wtot3], f32r, name=f"xt{k}", tag=f"xt{k}")
            gaps = xt[:, 0 : STRIDE * GROUP].rearrange("p (k c) -> p k c", c=STRIDE)
            dma_engines[k % len(dma_engines)].dma_start(
                gaps[:, :, 0:PAD0],
                z_d[:, 0 : 2 * GROUP].rearrange("p (k c) -> p k c", c=PAD0),
            )
            dma_engines[(k + 1) % len(dma_engines)].dma_start(
                xt[:, STRIDE * GROUP : wtot3], z_d[:, 6:8]
            )
            xts.append(xt)

        gs = _groups(n_images)
        chunks = [gs[s : s + qg] for s in range(0, len(gs), qg)]

        def emit_in_dmas(ci):
            for q, (i0, n) in enumerate(chunks[ci]):
                g = ci * qg + q
                xt = xts[g % xt_bufs]
                # one batched input DMA per group
                dst = xt[:, 0 : STRIDE * n].rearrange("p (k c) -> p k c", c=STRIDE)
                dma_engines[g % len(dma_engines)].dma_start(
                    dst[:, :, PAD0 : PAD0 + W],
                    x_d[i0 : i0 + n].rearrange("n h w -> h n w"),
                )

        emit_in_dmas(0)
        if len(chunks) > 1:
            emit_in_dmas(1)
        for ci, chunk in enumerate(chunks):
            # software-pipelined emission: prefetch two supergroups ahead so
            # PE never starves (HAM stays warm). xt_bufs = 3*qg keeps the
            # in-flight tiles distinct.
            if ci + 2 < len(chunks):
                emit_in_dmas(ci + 2)

            pts = [
                ppool.tile([H, STRIDE * n + 2], f32, tag="pt", name="pt")
                for (i0, n) in chunk
            ]
            # fp32r matmul ISA restrictions: dst start col even (8B-aligned)
            # and dst/src innermost lengths even. dst [2, 130n+2) (j<=2) /
            # [2, 130n) (j=3); extra columns land in zero-gap / never-read
            # psum columns, so correctness is unaffected. j-outer order so
            # the 4 weight loads amortize over qg matmuls each.
            for idx, j in enumerate((2, 0, 1, 3)):  # full-width write first
                d = j - PAD0
                for q, (i0, n) in enumerate(chunk):
                    a = PAD0
                    b = STRIDE * n + PAD0 - (PAD0 if d > 0 else 0)
                    nc.tensor.matmul(
                        pts[q][:, a:b],
                        wt[:, H * j : H * (j + 1)],
                        xts[(ci * qg + q) % xt_bufs][:, a + d : b + d],
                        start=(idx == 0),
                        stop=(idx == 3),
                    )

            for q, (i0, n) in enumerate(chunk):
                g = ci * qg + q
                pt = pts[q]
                ot = opool.tile([H, n * W], f32, tag="ot", name="ot")
                psrc = pt[:, 0 : STRIDE * n].rearrange("p (k c) -> p k c", c=STRIDE)
                odst = ot.rearrange("p (k c) -> p k c", c=W)
                if g % 2 == 0:
                    nc.vector.tensor_copy(odst, psrc[:, :, PAD0 : PAD0 + W])
                else:
                    nc.scalar.copy(odst, psrc[:, :, PAD0 : PAD0 + W])
                osrc = ot.rearrange("p (k c) -> p k c", c=W)
                dma_engines[(g + 1) % len(dma_engines)].dma_start(
                    y_d[i0 : i0 + n].rearrange("n h w -> h n w"), osrc
                )

    nc.compile()
    return nc


def _get_program(n_images: int):
    if n_images not in _PROGRAM_CACHE:
        _PROGRAM_CACHE[n_images] = build_program(n_images)
    return _PROGRAM_CACHE[n_images]


def kernel(x: np.ndarray, kernel: np.ndarray, _trace: bool = False):
    x = np.ascontiguousarray(x, dtype=np.float32)
    assert x.shape == (B, C, H, W), x.shape
    bands = _band_matrices(kernel)

    n_total = B * C
    n_per_core = n_total // N_CORES
    xf = x.reshape(n_total, H, W)

    nc = _get_program(n_per_core)
    in_maps = [
        {
            "x": _round_fp32r(xf[c * n_per_core : (c + 1) * n_per_core]),
            "bands": _round_fp32r(bands),
            "zeros": np.zeros((H, 2 * GROUP + 2), dtype=np.float32),
        }
        for c in range(N_CORES)
    ]
    res = bass_utils.run_bass_kernel_spmd(
        nc, in_maps, core_ids=list(range(N_CORES)), trace=_trace
    )
    y = np.concatenate([r["y"] for r in res.results], axis=0).reshape(B, C, H, W)
    if _trace:
        return y, res
    return y


# revision 17
# speedup vs baseline: 2.9790x; 1.0336x over previous
"""Trainium2 Bass kernel for nn_Blur: depthwise 4x4 FIR blur (upfirdn2d pad=(2,1)).

Full inputs in, full output out. Internally shards the 4096 (b,c) images
across 8 NeuronCores (pure data parallel, no collectives).

Algorithm (per core, 512 images of [H=128, W=128]):
  out[ho, wo] = sum_{i,j} wf[i, j] * x[ho+i-2, wo+j-2]   (wf = flipped kernel)
which factors into 4 column-convolutions along H, each a banded matmul with
the contraction over the partition (H) axis, with the W-shift (j-2) realized
as a shifted PSUM write of an accumulating matmul:
  psum[:, c] += W_j^T @ x[:, c + (j-2)]     W_j[hi, ho] = wf[hi-ho+2, j]
Images are packed 3-per-PSUM-bank with 2-column zero gaps so the shifted
reads pick up zero padding at image edges and the moving free dim (391)
stays >= 256, where float32r matmuls run at 1 cycle/row.
"""

import os
import sys
from contextlib import ExitStack

for _p in ("/opt/trn_rl_repo", "/root/.axon_site/_ro/trn_rl_repo"):
    if os.path.isdir(_p) and _p not in sys.path:
        sys.path.append(_p)

import numpy as np

import concourse.bass as bass  # noqa: F401  (engine types referenced via nc)
import concourse.tile as tile
from concourse import bacc, bass_utils, mybir

B, C, H, W = 16, 256, 128, 128
N_CORES = 8
GROUP = 3          # images per PSUM bank / matmul group
STRIDE = 130       # 2-col gap + 128 data cols per image in the packed tile
PAD0 = 2           # upfirdn2d pad before (both spatial dims)

_PROGRAM_CACHE: dict[int, object] = {}


def _round_fp32r(a: np.ndarray) -> np.ndarray:
    """Round fp32 to fp32r (11-bit mantissa, RNE) — matches neuronxcc's
    static_cast_fp32_to_fp32r bit-exactly."""
    u = np.ascontiguousarray(a, dtype=np.float32).view(np.uint32)
    r = (u.astype(np.uint64) + 0x800 - ((u >> 12) & 1)) & 0xFFFFF000
    return r.astype(np.uint32).view(np.float32)


def _band_matrices(kern: np.ndarray) -> np.ndarray:
    """bands[j][hi, ho] = wf[hi-ho+2, j], wf = flip(kern). Shape [4,128,128]."""
    wf = np.flip(np.asarray(kern, dtype=np.float64), (0, 1))
    bands = np.zeros((4, H, H), dtype=np.float64)
    ho = np.arange(H)
    for j in range(4):
        for i in range(4):
            d = i - PAD0            # hi - ho
            hi = ho + d
            m = (hi >= 0) & (hi < H)
            bands[j][hi[m], ho[m]] = wf[i, j]
    return np.ascontiguousarray(bands.astype(np.float32))


def _groups(n_images: int):
    out = []
    i = 0
    while i < n_images:
        n = min(GROUP, n_images - i)
        out.append((i, n))
        i += n
    # avoid a trailing 1-image group (moving dim 131 < 256 is 4x slower):
    # rebalance the last two groups 3+1 -> 2+2
    if len(out) >= 2 and out[-1][1] == 1:
        i0, n0 = out[-2]
        out[-2] = (i0, 2)
        out[-1] = (i0 + 2, 2)
    return out


def build_program(n_images: int, xt_bufs: int = 12, qg: int = 4):
    """Build + compile the per-core Bass program for n_images [128,128] images.

    qg = PSUM banks in flight per weight-batched supergroup (j-outer order
    keeps PE matmuls dense); DMAs round-robin across engine DGE rings.
    """
    nc = bacc.Bacc("TRN2", target_bir_lowering=False, debug=False)
    f32 = mybir.dt.float32
    f32r = mybir.dt.float32r

    x_d = nc.dram_tensor("x", [n_images, H, W], f32r, kind="ExternalInput")
    b_d = nc.dram_tensor("bands", [4, H, H], f32r, kind="ExternalInput")
    z_d = nc.dram_tensor("zeros", [H, 2 * GROUP + 2], f32r, kind="ExternalInput")
    y_d = nc.dram_tensor("y", [n_images, H, W], f32, kind="ExternalOutput")

    wtot3 = STRIDE * GROUP + 2  # even width; cols {130k, 130k+1} are zero gaps

    with ExitStack() as ctx:
        tc = ctx.enter_context(tile.TileContext(nc))
        wpool = ctx.enter_context(tc.tile_pool(name="wpool", bufs=1))
        xpool = ctx.enter_context(tc.tile_pool(name="xpool", bufs=1))
        opool = ctx.enter_context(tc.tile_pool(name="opool", bufs=8))
        ppool = ctx.enter_context(tc.tile_pool(name="ppool", bufs=8, space="PSUM"))

        dma_engines = [nc.sync, nc.scalar, nc.gpsimd]

        wt = wpool.tile([H, 4 * H], f32r)
        nc.sync.dma_start(
            wt.rearrange("p (j b) -> p j b", b=H), b_d.rearrange("j a b -> a j b")
        )

        # Persistent input tiles: gap columns are zeroed ONCE via DMA from the
        # zeros input; per-group DMAs only ever write data columns, so the
        # zero padding between images survives tile reuse. (A memset would be
        # simpler but fp32r memset fails walrus codegen / crashes the engine.)
        xts = []
        for k in range(xt_bufs):
            xt = xpool.tile([H, wtot3], f32r, name=f"xt{k}", tag=f"xt{k}")
            gaps = xt[:, 0 : STRIDE * GROUP].rearrange("p (k c) -> p k c", c=STRIDE)
            dma_engines[k % len(dma_engines)].dma_start(
                gaps[:, :, 0:PAD0],
                z_d[:, 0 : 2 * GROUP].rearrange("p (k c) -> p k c", c=PAD0),
            )
            dma_engines[(k + 1) % len(dma_engines)].dma_start(
                xt[:, STRIDE * GROUP : wtot3], z_d[:, 6:8]
            )
            xts.append(xt)

        gs = _groups(n_images)
        chunks = [gs[s : s + qg] for s in range(0, len(gs), qg)]

        def emit_in_dmas(ci):
            for q, (i0, n) in enumerate(chunks[ci]):
                g = ci * qg + q
                xt = xts[g % xt_bufs]
                # one batched input DMA per group
                dst = xt[:, 0 : STRIDE * n].rearrange("p (k c) -> p k c", c=STRIDE)
                dma_engines[g % len(dma_engines)].dma_start(
                    dst[:, :, PAD0 : PAD0 + W],
                    x_d[i0 : i0 + n].rearrange("n h w -> h n w"),
                )

        emit_in_dmas(0)
        if len(chunks) > 1:
            emit_in_dmas(1)
        for ci, chunk in enumerate(chunks):
            # software-pipelined emission: prefetch two supergroups ahead so
            # PE never starves (HAM stays warm). xt_bufs = 3*qg keeps the
            # in-flight tiles distinct.
            if ci + 2 < len(chunks):
                emit_in_dmas(ci + 2)

            pts = [
                ppool.tile([H, STRIDE * n + 2], f32, tag="pt", name="pt")
                for (i0, n) in chunk
            ]
            # fp32r matmul ISA restrictions: dst start col even (8B-aligned)
            # and dst/src innermost lengths even. dst [2, 130n+2) (j<=2) /
            # [2, 130n) (j=3); extra columns land in zero-gap / never-read
            # psum columns, so correctness is unaffected. j-outer order so
            # the 4 weight loads amortize over qg matmuls each.
            for idx, j in enumerate((2, 0, 1, 3)):  # full-width write first
                d = j - PAD0
                for q, (i0, n) in enumerate(chunk):
                    a = PAD0
                    b = STRIDE * n + PAD0 - (PAD0 if d > 0 else 0)
                    nc.tensor.matmul(
                        pts[q][:, a:b],
                        wt[:, H * j : H * (j + 1)],
                        xts[(ci * qg + q) % xt_bufs][:, a + d : b + d],
                        start=(idx == 0),
                        stop=(idx == 3),
                    )

            for q, (i0, n) in enumerate(chunk):
                g = ci * qg + q
                pt = pts[q]
                ot = opool.tile([H, n * W], f32, tag="ot", name="ot")
                psrc = pt[:, 0 : STRIDE * n].rearrange("p (k c) -> p k c", c=STRIDE)
                odst = ot.rearrange("p (k c) -> p k c", c=W)
                if g % 2 == 0:
                    nc.vector.tensor_copy(odst, psrc[:, :, PAD0 : PAD0 + W])
                else:
                    nc.scalar.copy(odst, psrc[:, :, PAD0 : PAD0 + W])
                osrc = ot.rearrange("p (k c) -> p k c", c=W)
                dma_engines[(g + 1) % len(dma_engines)].dma_start(
                    y_d[i0 : i0 + n].rearrange("n h w -> h n w"), osrc
                )

    nc.compile()
    return nc


def _get_program(n_images: int):
    if n_images not in _PROGRAM_CACHE:
        _PROGRAM_CACHE[n_images] = build_program(n_images)
    return _PROGRAM_CACHE[n_images]


def kernel(x: np.ndarray, kernel: np.ndarray, _trace: bool = False):
    x = np.ascontiguousarray(x, dtype=np.float32)
    assert x.shape == (B, C, H, W), x.shape
    bands = _band_matrices(kernel)

    n_total = B * C
    n_per_core = n_total // N_CORES
    xf = x.reshape(n_total, H, W)

    nc = _get_program(n_per_core)
    in_maps = [
        {
            "x": _round_fp32r(xf[c * n_per_core : (c + 1) * n_per_core]),
            "bands": _round_fp32r(bands),
            "zeros": np.zeros((H, 2 * GROUP + 2), dtype=np.float32),
        }
        for c in range(N_CORES)
    ]
    res = bass_utils.run_bass_kernel_spmd(
        nc, in_maps, core_ids=list(range(N_CORES)), trace=_trace
    )
    y = np.concatenate([r["y"] for r in res.results], axis=0).reshape(B, C, H, W)
    if _trace:
        return y, res
    return y
